# revision 6
# baseline (speedup 1.0000x reference)
"""Trainium2 Bass kernel for nn_CausalAttentionSortNet.

Math (per bh slice, reformulated as constant matmuls):
  sq[i, d] = (1/8) * (1/(64*i+1)) * sum_{t<=64*i} q[t, d]          = Aq @ q
  sk[j, d] = sum_{t in bucket j} cumsum(k)[t]/(t+1) summed weights  = Mk @ k
  Rc[i, j] = sum_d sq[i,d]*sk[j,d]                (= R[:, 1:], col 0 of R is 0)
  R masked where (col-1) >= row, then hard top-1 of softmax:
  out[i, jmax] = 1/sum_j exp(R[i,j]-max_j R), zero elsewhere.

Both Aq [64,4096] and Mk [64,4096] are data-independent, so the heavy part is
two streaming matmuls over q and k per bh (memory-bound). Sharding: bh axis
across 8 cores, 8 bh per core, zero communication.

On-chip layout per core: data tiles [128p, 2bh, 32r, 64d] with t = 32*p + r
(fully contiguous 1MB-per-bh DMAs). Matmul (per 32 chunks r): stationary
lhsT = data[:, :, r, :] (K=128, M=128=(bh,d)), moving rhs = W[:, r, :]
(N=64 summary rows), accumulated into PSUM [128, 64] = [(bh,d), i].
"""

import numpy as np

BH, SEQ, DIM = 64, 4096, 64
NCORES = 8
BH_PER_CORE = BH // NCORES
GROUPS = BH_PER_CORE // 2  # 2 bh per group
FLTMAX = float(np.finfo(np.float32).max)

_CACHE = {}


def _constants():
    t = np.arange(SEQ, dtype=np.float64)
    i = np.arange(64, dtype=np.float64)[:, None]
    # Aq[i, t] = 1/(8*(64i+1)) for t <= 64i else 0   (includes the dim^-0.5 = 1/8)
    aq = np.where(t[None, :] <= 64 * i, 1.0 / (8.0 * (64 * i + 1.0)), 0.0)
    # Mk[j, t]: weight of k[t] in sk[j] = sum over bucket-j of cumavg
    inv = 1.0 / (t + 1.0)
    invb = inv.reshape(64, 64)
    suffix = np.cumsum(invb[:, ::-1], axis=1)[:, ::-1]  # suffix[j, s] = sum_{u>=s} 1/(64j+u+1)
    cj = invb.sum(axis=1)
    mk = np.zeros((64, SEQ))
    for j in range(64):
        mk[j, : 64 * j] = cj[j]
        mk[j, 64 * j : 64 * j + 64] = suffix[j]
    # SBUF weight layout [p, r, i] with t = 32p + r
    wq = aq.T.reshape(128, 32, 64).astype(np.float32)
    wk = mk.T.reshape(128, 32, 64).astype(np.float32)
    wq = np.ascontiguousarray(wq)
    wk = np.ascontiguousarray(wk)
    # additive causal mask on R[:, 1:]: masked where jc >= i
    maskadd = np.where(
        np.arange(64)[None, :] >= np.arange(64)[:, None], -FLTMAX, 0.0
    ).astype(np.float32)
    return wq, wk, maskadd


def _build_nc(reps=1):
    from contextlib import ExitStack

    import concourse.bacc as bacc
    import concourse.mybir as mybir
    import concourse.tile as tile

    f32 = mybir.dt.float32
    wq_np, wk_np, mask_np = _constants()

    nc = bacc.Bacc(trn_type="TRN2")
    q = nc.dram_tensor("q", [BH_PER_CORE, SEQ, DIM], f32, kind="ExternalInput")
    k = nc.dram_tensor("k", [BH_PER_CORE, SEQ, DIM], f32, kind="ExternalInput")
    out = nc.dram_tensor("out", [BH_PER_CORE, 64, 65], f32, kind="ExternalOutput")
    wq_dram = nc.inline_tensor(wq_np, "wq_const")
    wk_dram = nc.inline_tensor(wk_np, "wk_const")
    mask_dram = nc.inline_tensor(mask_np, "mask_const")

    q_ap, k_ap, out_ap = q.ap(), k.ap(), out.ap()

    with tile.TileContext(nc) as tc, ExitStack() as ctx:
        singles = ctx.enter_context(tc.tile_pool(name="singles", bufs=1))
        data = ctx.enter_context(tc.tile_pool(name="data", bufs=2))
        small = ctx.enter_context(tc.tile_pool(name="small", bufs=3))
        psum = ctx.enter_context(tc.tile_pool(name="psum", bufs=2, space="PSUM"))
        rpsum = ctx.enter_context(tc.tile_pool(name="rpsum", bufs=2, space="PSUM"))

        wq_sb = singles.tile([128, 32, 64], f32)
        wk_sb = singles.tile([128, 32, 64], f32)
        mask_sb = singles.tile([64, 64], f32)
        nc.sync.dma_start(wq_sb[:], wq_dram.ap())
        nc.sync.dma_start(wk_sb[:], wk_dram.ap())
        nc.sync.dma_start(mask_sb[:], mask_dram.ap())

        for rep_g in range(reps * GROUPS):
            g = rep_g % GROUPS
            qt = data.tile([128, 2, 32, 64], f32, tag="qt")
            kt = data.tile([128, 2, 32, 64], f32, tag="kt")
            nc.sync.dma_start(
                qt[:], q_ap[2 * g : 2 * g + 2].rearrange("b (p r) d -> p b r d", p=128)
            )
            nc.sync.dma_start(
                kt[:], k_ap[2 * g : 2 * g + 2].rearrange("b (p r) d -> p b r d", p=128)
            )
            psq = psum.tile([128, 64], f32, tag="psq")
            psk = psum.tile([128, 64], f32, tag="psk")
            # Stationary (weights) APs must be 2D [K, M] for walrus, so one
            # matmul per bh half: out partitions 64b..64b+64 of the PSUM tile.
            for b in range(2):
                for r in range(32):
                    nc.tensor.matmul(
                        psq[64 * b : 64 * b + 64, :],
                        lhsT=qt[:, b, r, :], rhs=wq_sb[:, r, :],
                        start=(r == 0), stop=(r == 31),
                    )
            for b in range(2):
                for r in range(32):
                    nc.tensor.matmul(
                        psk[64 * b : 64 * b + 64, :],
                        lhsT=kt[:, b, r, :], rhs=wk_sb[:, r, :],
                        start=(r == 0), stop=(r == 31),
                    )
            sq_sb = small.tile([128, 64], f32, tag="sq")
            sk_sb = small.tile([128, 64], f32, tag="sk")
            nc.vector.tensor_copy(sq_sb[:], psq[:])
            nc.vector.tensor_copy(sk_sb[:], psk[:])
            for b in range(2):
                bh = 2 * g + b
                pr = rpsum.tile([64, 64], f32, tag="pr")
                nc.tensor.matmul(
                    pr[:],
                    lhsT=sq_sb[64 * b : 64 * b + 64, :],
                    rhs=sk_sb[64 * b : 64 * b + 64, :],
                    start=True, stop=True,
                )
                rf = small.tile([64, 65], f32, tag="rf")
                nc.vector.memset(rf[:, 0:1], 0.0)
                nc.vector.tensor_add(rf[:, 1:65], pr[:], mask_sb[:])
                m = small.tile([64, 1], f32, tag="m")
                nm = small.tile([64, 1], f32, tag="nm")
                s = small.tile([64, 1], f32, tag="s")
                rr = small.tile([64, 1], f32, tag="rr")
                nc.vector.reduce_max(m[:], rf[:], axis=mybir.AxisListType.X)
                nc.vector.tensor_scalar_mul(nm[:], m[:], -1.0)
                e = small.tile([64, 65], f32, tag="e")
                nc.scalar.activation(
                    e[:], rf[:], mybir.ActivationFunctionType.Exp,
                    bias=nm[:], scale=1.0, accum_out=s[:],
                )
                nc.vector.reciprocal(rr[:], s[:])
                o = small.tile([64, 65], f32, tag="o")
                nc.vector.tensor_scalar(
                    out=o[:], in0=rf[:], scalar1=m[:], scalar2=rr[:],
                    op0=mybir.AluOpType.is_equal, op1=mybir.AluOpType.mult,
                )
                nc.sync.dma_start(out_ap[bh], o[:])

    nc.compile()
    return nc


def _get_nc(reps=1):
    key = ("nc", reps)
    if key not in _CACHE:
        _CACHE[key] = _build_nc(reps)
    return _CACHE[key]


def _make_runner(nc):
    """Persistent jit(shard_map) callable over the 8 cores for one Bass module.

    One function object per nc so jax.jit's cache is reused across calls
    (run_bass_kernel_spmd re-traces on every invocation)."""
    import jax
    from jax.sharding import Mesh, PartitionSpec
    from jax.experimental.shard_map import shard_map

    import concourse.mybir as mybir
    from concourse.bass2jax import (
        _bass_exec_p,
        install_neuronx_cc_hook,
        partition_id_tensor,
    )

    install_neuronx_cc_hook()

    partition_name = nc.partition_id_tensor.name if nc.partition_id_tensor else None
    in_names, out_names, out_avals, zero_shapes = [], [], [], []
    for alloc in nc.m.functions[0].allocations:
        if not isinstance(alloc, mybir.MemoryLocationSet):
            continue
        name = alloc.memorylocations[0].name
        if alloc.kind == "ExternalInput":
            if name != partition_name:
                in_names.append(name)
        elif alloc.kind == "ExternalOutput":
            out_names.append(name)
            shape = tuple(alloc.tensor_shape)
            dtype = mybir.dt.np(alloc.dtype)
            out_avals.append(jax.core.ShapedArray(shape, dtype))
            zero_shapes.append((shape, dtype))
    n_params = len(in_names)
    n_outs = len(out_avals)
    all_in_names = tuple(
        in_names + out_names + ([partition_name] if partition_name else [])
    )

    def _body(*args):
        operands = list(args)
        if partition_name is not None:
            operands.append(partition_id_tensor())
        return tuple(
            _bass_exec_p.bind(
                *operands,
                out_avals=tuple(out_avals),
                in_names=all_in_names,
                out_names=tuple(out_names),
                lowering_input_output_aliases=(),
                sim_require_finite=True,
                sim_require_nnan=True,
                nc=nc,
            )
        )

    devices = jax.devices()[:NCORES]
    mesh = Mesh(np.asarray(devices), ("core",))
    fn = jax.jit(
        shard_map(
            _body,
            mesh=mesh,
            in_specs=(PartitionSpec("core"),) * (n_params + n_outs),
            out_specs=(PartitionSpec("core"),) * n_outs,
            check_rep=False,
        ),
        donate_argnums=tuple(range(n_params, n_params + n_outs)),
        keep_unused=True,
    )

    name_to_idx = {n: i for i, n in enumerate(in_names)}
    out_idx = out_names.index("out")

    def run(q, k):
        import jax as _jax

        ins = [None] * n_params
        ins[name_to_idx["q"]] = q
        ins[name_to_idx["k"]] = k
        zeros = [
            np.zeros((NCORES * s[0], *s[1:]), dt) for (s, dt) in zero_shapes
        ]
        outs = fn(*ins, *zeros)
        _jax.block_until_ready(outs)
        return np.asarray(outs[out_idx]).reshape(BH, 64, 65)

    return run


def _get_runner(reps=1):
    key = ("runner", reps)
    if key not in _CACHE:
        _CACHE[key] = _make_runner(_get_nc(reps))
    return _CACHE[key]


def _prep(q, k):
    q = np.ascontiguousarray(np.asarray(q), dtype=np.float32)
    k = np.ascontiguousarray(np.asarray(k), dtype=np.float32)
    return q, k


def _run_spmd(q, k, trace=False, **kwargs):
    q, k = _prep(q, k)
    out = _get_runner(1)(q, k)
    return out, None


def kernel(q, k, topk=1):
    q, k = _prep(q, k)
    return _get_runner(1)(q, k)


# revision 8
# speedup vs baseline: 499.0160x; 499.0160x over previous
"""Trainium2 Bass kernel for nn_CausalAttentionSortNet.

Math (per bh slice, reformulated as constant matmuls):
  sq[i, d] = (1/8) * (1/(64*i+1)) * sum_{t<=64*i} q[t, d]          = Aq @ q
  sk[j, d] = sum_{t in bucket j} cumsum(k)[t]/(t+1) summed weights  = Mk @ k
  Rc[i, j] = sum_d sq[i,d]*sk[j,d]                (= R[:, 1:], col 0 of R is 0)
  R masked where (col-1) >= row, then hard top-1 of softmax:
  out[i, jmax] = 1/sum_j exp(R[i,j]-max_j R), zero elsewhere.

Both Aq [64,4096] and Mk [64,4096] are data-independent, so the heavy part is
two streaming matmuls over q and k per bh (memory-bound). Sharding: bh axis
across 8 cores, 8 bh per core, zero communication.

On-chip layout per core: data tiles [128p, 2bh, 32r, 64d] with t = 32*p + r
(fully contiguous 1MB-per-bh DMAs). Matmul (per 32 chunks r): stationary
lhsT = data[:, :, r, :] (K=128, M=128=(bh,d)), moving rhs = W[:, r, :]
(N=64 summary rows), accumulated into PSUM [128, 64] = [(bh,d), i].
"""

import numpy as np

BH, SEQ, DIM = 64, 4096, 64
NCORES = 8
BH_PER_CORE = BH // NCORES
GROUPS = BH_PER_CORE // 2  # 2 bh per group
FLTMAX = float(np.finfo(np.float32).max)

_CACHE = {}


def _constants():
    t = np.arange(SEQ, dtype=np.float64)
    i = np.arange(64, dtype=np.float64)[:, None]
    # Aq[i, t] = 1/(8*(64i+1)) for t <= 64i else 0   (includes the dim^-0.5 = 1/8)
    aq = np.where(t[None, :] <= 64 * i, 1.0 / (8.0 * (64 * i + 1.0)), 0.0)
    # Mk[j, t]: weight of k[t] in sk[j] = sum over bucket-j of cumavg
    inv = 1.0 / (t + 1.0)
    invb = inv.reshape(64, 64)
    suffix = np.cumsum(invb[:, ::-1], axis=1)[:, ::-1]  # suffix[j, s] = sum_{u>=s} 1/(64j+u+1)
    cj = invb.sum(axis=1)
    mk = np.zeros((64, SEQ))
    for j in range(64):
        mk[j, : 64 * j] = cj[j]
        mk[j, 64 * j : 64 * j + 64] = suffix[j]
    # SBUF weight layout [p, r, i] with t = 32p + r
    wq = aq.T.reshape(128, 32, 64).astype(np.float32)
    wk = mk.T.reshape(128, 32, 64).astype(np.float32)
    wq = np.ascontiguousarray(wq)
    wk = np.ascontiguousarray(wk)
    # additive causal mask on R[:, 1:]: masked where jc >= i
    maskadd = np.where(
        np.arange(64)[None, :] >= np.arange(64)[:, None], -FLTMAX, 0.0
    ).astype(np.float32)
    return wq, wk, maskadd


def _build_nc(reps=1):
    from contextlib import ExitStack

    import concourse.bacc as bacc
    import concourse.mybir as mybir
    import concourse.tile as tile

    f32 = mybir.dt.float32
    wq_np, wk_np, mask_np = _constants()

    nc = bacc.Bacc(trn_type="TRN2")
    q = nc.dram_tensor("q", [BH_PER_CORE, SEQ, DIM], f32, kind="ExternalInput")
    k = nc.dram_tensor("k", [BH_PER_CORE, SEQ, DIM], f32, kind="ExternalInput")
    out = nc.dram_tensor("out", [BH_PER_CORE, 64, 65], f32, kind="ExternalOutput")
    wq_dram = nc.inline_tensor(wq_np, "wq_const")
    wk_dram = nc.inline_tensor(wk_np, "wk_const")
    mask_dram = nc.inline_tensor(mask_np, "mask_const")

    q_ap, k_ap, out_ap = q.ap(), k.ap(), out.ap()

    with tile.TileContext(nc) as tc, ExitStack() as ctx:
        singles = ctx.enter_context(tc.tile_pool(name="singles", bufs=1))
        data = ctx.enter_context(tc.tile_pool(name="data", bufs=2))
        small = ctx.enter_context(tc.tile_pool(name="small", bufs=3))
        psum = ctx.enter_context(tc.tile_pool(name="psum", bufs=2, space="PSUM"))
        rpsum = ctx.enter_context(tc.tile_pool(name="rpsum", bufs=2, space="PSUM"))

        wq_sb = singles.tile([128, 32, 64], f32)
        wk_sb = singles.tile([128, 32, 64], f32)
        mask_sb = singles.tile([64, 64], f32)
        nc.sync.dma_start(wq_sb[:], wq_dram.ap())
        nc.sync.dma_start(wk_sb[:], wk_dram.ap())
        nc.sync.dma_start(mask_sb[:], mask_dram.ap())

        for rep_g in range(reps * GROUPS):
            g = rep_g % GROUPS
            qt = data.tile([128, 2, 32, 64], f32, tag="qt")
            kt = data.tile([128, 2, 32, 64], f32, tag="kt")
            nc.sync.dma_start(
                qt[:], q_ap[2 * g : 2 * g + 2].rearrange("b (p r) d -> p b r d", p=128)
            )
            nc.sync.dma_start(
                kt[:], k_ap[2 * g : 2 * g + 2].rearrange("b (p r) d -> p b r d", p=128)
            )
            psq = psum.tile([128, 64], f32, tag="psq")
            psk = psum.tile([128, 64], f32, tag="psk")
            # Stationary (weights) APs must be 2D [K, M] for walrus, so one
            # matmul per bh half: out partitions 64b..64b+64 of the PSUM tile.
            for b in range(2):
                for r in range(32):
                    nc.tensor.matmul(
                        psq[64 * b : 64 * b + 64, :],
                        lhsT=qt[:, b, r, :], rhs=wq_sb[:, r, :],
                        start=(r == 0), stop=(r == 31),
                    )
            for b in range(2):
                for r in range(32):
                    nc.tensor.matmul(
                        psk[64 * b : 64 * b + 64, :],
                        lhsT=kt[:, b, r, :], rhs=wk_sb[:, r, :],
                        start=(r == 0), stop=(r == 31),
                    )
            sq_sb = small.tile([128, 64], f32, tag="sq")
            sk_sb = small.tile([128, 64], f32, tag="sk")
            nc.vector.tensor_copy(sq_sb[:], psq[:])
            nc.vector.tensor_copy(sk_sb[:], psk[:])
            for b in range(2):
                bh = 2 * g + b
                pr = rpsum.tile([64, 64], f32, tag="pr")
                nc.tensor.matmul(
                    pr[:],
                    lhsT=sq_sb[64 * b : 64 * b + 64, :],
                    rhs=sk_sb[64 * b : 64 * b + 64, :],
                    start=True, stop=True,
                )
                rf = small.tile([64, 65], f32, tag="rf")
                nc.vector.memset(rf[:, 0:1], 0.0)
                nc.vector.tensor_add(rf[:, 1:65], pr[:], mask_sb[:])
                m = small.tile([64, 1], f32, tag="m")
                nm = small.tile([64, 1], f32, tag="nm")
                s = small.tile([64, 1], f32, tag="s")
                rr = small.tile([64, 1], f32, tag="rr")
                nc.vector.reduce_max(m[:], rf[:], axis=mybir.AxisListType.X)
                nc.vector.tensor_scalar_mul(nm[:], m[:], -1.0)
                e = small.tile([64, 65], f32, tag="e")
                nc.scalar.activation(
                    e[:], rf[:], mybir.ActivationFunctionType.Exp,
                    bias=nm[:], scale=1.0, accum_out=s[:],
                )
                nc.vector.reciprocal(rr[:], s[:])
                o = small.tile([64, 65], f32, tag="o")
                nc.vector.tensor_scalar(
                    out=o[:], in0=rf[:], scalar1=m[:], scalar2=rr[:],
                    op0=mybir.AluOpType.is_equal, op1=mybir.AluOpType.mult,
                )
                nc.sync.dma_start(out_ap[bh], o[:])

    nc.compile()
    nc._kern_reps = reps
    return nc


def _get_nc(reps=1):
    key = ("nc", reps)
    if key not in _CACHE:
        _CACHE[key] = _build_nc(reps)
    return _CACHE[key]


def _make_runner(nc):
    """Persistent jit(shard_map) callable over the 8 cores for one Bass module.

    One function object per nc so jax.jit's cache is reused across calls
    (run_bass_kernel_spmd re-traces on every invocation)."""
    import jax
    from jax.sharding import Mesh, PartitionSpec
    from jax.experimental.shard_map import shard_map

    import concourse.mybir as mybir
    from concourse.bass2jax import (
        _bass_exec_p,
        install_neuronx_cc_hook,
        partition_id_tensor,
    )

    install_neuronx_cc_hook()

    partition_name = nc.partition_id_tensor.name if nc.partition_id_tensor else None
    in_names, out_names, out_avals, zero_shapes = [], [], [], []
    for alloc in nc.m.functions[0].allocations:
        if not isinstance(alloc, mybir.MemoryLocationSet):
            continue
        name = alloc.memorylocations[0].name
        if alloc.kind == "ExternalInput":
            if name != partition_name:
                in_names.append(name)
        elif alloc.kind == "ExternalOutput":
            out_names.append(name)
            shape = tuple(alloc.tensor_shape)
            dtype = mybir.dt.np(alloc.dtype)
            out_avals.append(jax.core.ShapedArray(shape, dtype))
            zero_shapes.append((shape, dtype))
    n_params = len(in_names)
    n_outs = len(out_avals)
    all_in_names = tuple(
        in_names + out_names + ([partition_name] if partition_name else [])
    )

    def _body(*args):
        operands = list(args)
        if partition_name is not None:
            operands.append(partition_id_tensor())
        return tuple(
            _bass_exec_p.bind(
                *operands,
                out_avals=tuple(out_avals),
                in_names=all_in_names,
                out_names=tuple(out_names),
                lowering_input_output_aliases=(),
                sim_require_finite=True,
                sim_require_nnan=True,
                nc=nc,
            )
        )

    devices = jax.devices()[:NCORES]
    mesh = Mesh(np.asarray(devices), ("core",))
    _CACHE[("runner_mesh", getattr(nc, "_kern_reps", 1))] = mesh
    fn = jax.jit(
        shard_map(
            _body,
            mesh=mesh,
            in_specs=(PartitionSpec("core"),) * (n_params + n_outs),
            out_specs=(PartitionSpec("core"),) * n_outs,
            check_rep=False,
        ),
        donate_argnums=tuple(range(n_params, n_params + n_outs)),
        keep_unused=True,
    )

    name_to_idx = {n: i for i, n in enumerate(in_names)}
    out_idx = out_names.index("out")

    def run(q, k):
        import jax as _jax

        ins = [None] * n_params
        ins[name_to_idx["q"]] = q
        ins[name_to_idx["k"]] = k
        zeros = [
            np.zeros((NCORES * s[0], *s[1:]), dt) for (s, dt) in zero_shapes
        ]
        outs = fn(*ins, *zeros)
        _jax.block_until_ready(outs)
        return np.asarray(outs[out_idx]).reshape(BH, 64, 65)

    return run


def _get_runner(reps=1):
    key = ("runner", reps)
    if key not in _CACHE:
        _CACHE[key] = _make_runner(_get_nc(reps))
    return _CACHE[key]


def _prep(q, k):
    q = np.ascontiguousarray(np.asarray(q), dtype=np.float32)
    k = np.ascontiguousarray(np.asarray(k), dtype=np.float32)
    return q, k


def _run_spmd(q, k, trace=False, **kwargs):
    q, k = _prep(q, k)
    out = _get_runner(1)(q, k)
    return out, None


def kernel(q, k, topk=1):
    q, k = _prep(q, k)
    return _get_runner(1)(q, k)
